# revision 48
# baseline (speedup 1.0000x reference)
"""Trainium2 Bass kernel for MatrixOdeGradientDescentModel.

Reference computation (B=4096, DZ=512, H=2048, DY=10, n_steps=64):
    z = x; repeat n_steps: z += dt * z @ A.T          (dt = 1/n_steps)
    y = relu(z @ W1.T + b1) @ W2.T + b2

Algebraic rewrite: the Euler loop is linear, so
    z_final = x @ P^T with P^T = (W)^n,  W = I + dt*A^T  (T0 := dt*A^T).
(W)^n = sum_k C(n,k) T0^k. Since ||T0|| = ||A||/n (~0.014 here), the series
truncated at degree 9 has ~1e-7 relative tail for any n (C(n,k)/n^k <= 1/k!),
so we evaluate it Paterson-Stockmeyer style with X = T0^2:
    P_dev = c1*T0 + X*(B1 + X*(B2 + X*(B3 + X*B4)))     [P = I + P_dev]
where B_j = c_{2j}*I + c_{2j+1}*T0 are built on the DVE (off the PE's
critical path) and folded into the PSUM evictions. X enters products only as
its transpose D0^2 (D0 := T0^T, built by PE transposes against the identity
while the input DMAs stream). Then zT = xT + P_dev-apply(xT), and the MLP.

Sharding: data-parallel over batch. Each of the 8 cores gets 512 rows of x;
A/W1/W2 replicated; no cross-core communication.

Matmuls run in float32r (TF32-like, 4x faster than fp32 on the PE) with fp32
PSUM accumulation; the identity-free deviation formulation keeps the
end-to-end relative error at the ~2e-4 level.
"""

import os
from math import comb

import numpy as np

import concourse.bacc as bacc
import concourse.mybir as mybir
import concourse.tile as tile
from concourse.bass_utils import run_bass_kernel_spmd

P = 128
B, DZ, H, DY = 4096, 512, 2048, 10
NCORES = 8
BC = B // NCORES          # 512 rows per core
DT = DZ // P              # 4 k-tiles over DZ
HT = H // P               # 16 m-tiles over H

f32 = mybir.dt.float32
f32r = mybir.dt.float32r

_BUILD_CACHE = {}


def _emit_mm_set(nc, psum_pool, lhsT_tile, rhs_tile, evict, n_mt=DT,
                 kt_major=False):
    """One [512,512]-ish matmul set. mt-major (default) evicts each PSUM as
    soon as its k-accumulation finishes, freeing slots early. kt-major runs
    all n_mt PSUM accumulations in parallel so the k-th matmul burst only
    needs the k-th input tiles — right when a set's inputs trickle in from
    DMA or a producer's staggered evictions."""
    if kt_major:
        pss = [psum_pool.tile([P, BC], f32, tag="ps", name=f"ps{mt}")
               for mt in range(n_mt)]
        for kt in range(DT):
            for mt in range(n_mt):
                nc.tensor.matmul(
                    pss[mt][:],
                    lhsT_tile[:, kt, mt * P:(mt + 1) * P],
                    rhs_tile[:, kt, :],
                    start=(kt == 0),
                    stop=(kt == DT - 1),
                )
        for mt in range(n_mt):
            evict(mt, pss[mt])
        return
    for mt in range(n_mt):
        ps = psum_pool.tile([P, BC], f32, tag="ps")
        for kt in range(DT):
            nc.tensor.matmul(
                ps[:],
                lhsT_tile[:, kt, mt * P:(mt + 1) * P],
                rhs_tile[:, kt, :],
                start=(kt == 0),
                stop=(kt == DT - 1),
            )
        evict(mt, ps)


def _build(n_steps: int):
    """Build + compile the Bass module for a given n_steps."""
    n = int(n_steps)
    assert n >= 0
    nc = bacc.Bacc("TRN2", target_bir_lowering=False, debug=False,
                   enable_asserts=False, num_devices=NCORES)

    # f32r-declared DRAM inputs carry raw fp32 bytes; the PE rounds internally
    # (verified bit-identical to an explicit cast) so plain HWDGE DMA works.
    xt_d = nc.dram_tensor("xt", [P, DT * BC], f32, kind="ExternalInput")
    xtr_d = nc.dram_tensor("xtr", [P, DT * BC], f32r, kind="ExternalInput")
    t0_d = nc.dram_tensor("t0", [P, DT * DZ], f32r, kind="ExternalInput")
    w1t_d = nc.dram_tensor("w1t", [P, DT * H], f32r, kind="ExternalInput")
    b1t_d = nc.dram_tensor("b1t", [P, HT], f32, kind="ExternalInput")
    w2t_d = nc.dram_tensor("w2t", [P, HT * DY], f32r, kind="ExternalInput")
    b2t_d = nc.dram_tensor("b2t", [DY, 1], f32, kind="ExternalInput")
    ident_d = nc.dram_tensor("ident", [P, P], f32, kind="ExternalInput")
    identr_d = nc.dram_tensor("identr", [P, P], f32r, kind="ExternalInput")
    y_d = nc.dram_tensor("y", [BC, DY], f32, kind="ExternalOutput")

    mult = mybir.AluOpType.mult
    add = mybir.AluOpType.add
    c = [float(comb(n, k)) for k in range(10)]

    with tile.TileContext(nc) as tc:
        with (
            tc.tile_pool(name="const", bufs=1) as const_pool,
            tc.tile_pool(name="weights", bufs=1) as w_pool,
            tc.tile_pool(name="horner", bufs=2) as horner_pool,
            tc.tile_pool(name="bpool", bufs=2) as b_pool,
            tc.tile_pool(name="accp", bufs=2) as acc_pool,
            tc.tile_pool(name="acts", bufs=1) as act_pool,
            tc.tile_pool(name="out", bufs=2) as out_pool,
            tc.tile_pool(name="psum", bufs=7, space="PSUM") as psum_pool,
            tc.tile_pool(name="psum_y", bufs=1, space="PSUM") as psum_y_pool,
        ):
            # ---- loads: one HWDGE trigger queue, strict priority order -----
            # (DMA rings are FIFO and the two cores of an HBM stack share
            # ~350 GB/s, so chain-critical bytes must be enqueued first.)
            identr = const_pool.tile([P, P], f32r, tag="identr")
            nc.sync.dma_start(identr[:], identr_d.ap())
            t_cur = w_pool.tile([P, DT, DZ], f32r, tag="t0")
            t0_src = t0_d.ap().rearrange("p (t b) -> p t b", t=DT)
            for kt in range(DT):
                nc.sync.dma_start(t_cur[:, kt:kt + 1, :], t0_src[:, kt:kt + 1, :])

            def load(dram, shape, tag, dtype=f32r, chunks=1):
                r = w_pool.tile(shape, dtype, tag=tag)
                src = dram.ap().rearrange("p (t b) -> p t b", t=shape[1])
                for ch in range(chunks):
                    lo = shape[1] * ch // chunks
                    hi = shape[1] * (ch + 1) // chunks
                    nc.sync.dma_start(r[:, lo:hi, :], src[:, lo:hi, :])
                return r

            xt_r = load(xtr_d, [P, DT, BC], "xtr")
            xt = load(xt_d, [P, DT, BC], "xt", dtype=f32)
            w1t = load(w1t_d, [P, DT, H], "w1t", chunks=4)
            w2t = load(w2t_d, [P, HT, DY], "w2t")

            b1t = const_pool.tile([P, HT], f32, tag="b1t")
            nc.sync.dma_start(b1t[:], b1t_d.ap())
            b2t = const_pool.tile([DY, 1], f32, tag="b2t")
            nc.sync.dma_start(b2t[:], b2t_d.ap())
            ident = const_pool.tile([P, P], f32, tag="ident")
            nc.sync.dma_start(ident[:], ident_d.ap())

            # Brief PE warm-up while the t0 DMA streams: HAM only unthrottles
            # (1.2 -> 2.4 GHz) after ~3.4us of sustained matmul activity.
            ps_w0 = psum_y_pool.tile([P, P], f32, tag="psy")
            ps_w1 = psum_pool.tile([P, P], f32, tag="ps")
            for i in range(8):
                nc.tensor.matmul([ps_w0, ps_w1][i % 2][:], identr[:], identr[:],
                                 start=True, stop=True)

            # ---- D0 = T0^T via PE matmuls against the identity -------------
            # (saves a 1 MiB load on the DMA-critical front; also warms HAM)
            d_cur = w_pool.tile([P, DT, DZ], f32r, tag="d0")
            for a in range(DT):
                ps = psum_pool.tile([P, DZ], f32, tag="ps")
                for b in range(DT):
                    nc.tensor.matmul(
                        ps[:, b * P:(b + 1) * P],
                        t_cur[:, b, a * P:(a + 1) * P], identr[:],
                        start=True, stop=True)
                nc.scalar.activation(
                    d_cur[:, a, :], ps[:], mybir.ActivationFunctionType.Copy)

            # ---- scaled-diagonal helper (one reusable c*I big tile) --------
            cIbig = w_pool.tile([P, DT, DZ], f32, tag="cIbig")
            nc.gpsimd.memset(cIbig[:], 0.0)

            def set_diag(cv):
                for mt in range(DT):
                    nc.vector.tensor_scalar_mul(
                        cIbig[:, mt, mt * P:(mt + 1) * P], identr[:], cv)

            def make_b(cv_i, cv_t, dtype, tag, engine=None):
                """B = cv_i * I + cv_t * T0, built off the PE's critical path.
                f32 blocks go to GpSimd so the DVE stays free for evictions."""
                set_diag(cv_i)
                bt = b_pool.tile([P, DT, DZ], dtype, tag=tag)
                eng = engine if engine is not None else nc.vector
                eng.scalar_tensor_tensor(
                    bt[:], t_cur[:], cv_t, cIbig[:], op0=mult, op1=add)
                return bt

            acc = xt_r          # zT accumulator, fp32r [P, DT, BC]
            acc_f32 = xt        # exact fp32 twin for the fused +acc add

            def apply_T(t_tile, acc_r, acc_exact):
                """acc <- acc + P_dev-rows @ acc."""
                new_r = acc_pool.tile([P, DT, BC], f32r, tag="acc")

                def evict(mt, ps):
                    nc.vector.scalar_tensor_tensor(
                        new_r[:, mt, :], acc_exact[:, mt, :], 1.0, ps[:],
                        op0=mult, op1=add)

                _emit_mm_set(nc, psum_pool, t_tile, acc_r, evict)
                return new_r

            if n == 0:
                zt = xt_r
            elif n == 1:
                zt = apply_T(t_cur, acc, acc_f32)
            else:
                # ---- Paterson-Stockmeyer, X = T0^2, degree 9 --------------
                # Y4 first (needed earliest as the first Horner rhs).
                y4t = make_b(c[8], c[9], f32r, "y4")

                # X as its transpose D0^2 (the lhsT for X-products).
                x2 = w_pool.tile([P, DT, DZ], f32r, tag="x2")

                def evict_x2(mt, ps):
                    nc.vector.tensor_copy(x2[:, mt, :], ps[:])

                _emit_mm_set(nc, psum_pool, t_cur, d_cur, evict_x2)

                # Horner levels: Y_j = B_j + X @ Y_{j+1}.
                y_r = y4t
                for j in (3, 2, 1):
                    bj = make_b(c[2 * j], c[2 * j + 1], f32, "bj")
                    ynew = horner_pool.tile([P, DT, DZ], f32r, tag="ylev")

                    def evict_y(mt, ps, ynew=ynew, bj=bj):
                        nc.vector.scalar_tensor_tensor(
                            ynew[:, mt, :], bj[:, mt, :], 1.0, ps[:],
                            op0=mult, op1=add)

                    _emit_mm_set(nc, psum_pool, x2, y_r, evict_y)
                    y_r = ynew

                # P_dev = c1*T0 + X @ Y1  (c1 = n)
                pd = w_pool.tile([P, DT, DZ], f32r, tag="pd")

                def evict_pd(mt, ps):
                    nc.vector.scalar_tensor_tensor(
                        pd[:, mt, :], t_cur[:, mt, :], c[1], ps[:],
                        op0=mult, op1=add)

                _emit_mm_set(nc, psum_pool, x2, y_r, evict_pd)

                # zT = xT + P_dev-rows @ xT
                zt = apply_T(pd, acc, acc_f32)

            # ---- MLP: hT = relu(W1 @ z + b1); yT = W2 @ h + b2 -------------
            # Layer-2 accumulation MMs interleave with layer-1 so the tail
            # after the last h-tile is just one MM + bias + transpose.
            ht = act_pool.tile([P, HT, BC], f32r, tag="ht")
            ps_y = psum_y_pool.tile([DY, BC], f32, tag="psy")
            for mt in range(HT):
                ps = psum_pool.tile([P, BC], f32, tag="ps")
                for kt in range(DT):
                    nc.tensor.matmul(
                        ps[:], w1t[:, kt, mt * P:(mt + 1) * P], zt[:, kt, :],
                        start=(kt == 0), stop=(kt == DT - 1))
                nc.scalar.activation(
                    ht[:, mt, :], ps[:], mybir.ActivationFunctionType.Relu,
                    bias=b1t[:, mt:mt + 1])
                nc.tensor.matmul(ps_y[:], w2t[:, mt, :], ht[:, mt, :],
                                 start=(mt == 0), stop=(mt == HT - 1))
            ytb = out_pool.tile([DY, BC], f32, tag="ytb")

            # ---- transpose yT -> y and store (evict chunked per b-block so
            # each transpose starts as soon as its columns are biased) -------
            y_sb = out_pool.tile([P, BC // P, DY], f32, tag="ysb")
            for bt in range(BC // P):
                nc.scalar.activation(ytb[:, bt * P:(bt + 1) * P],
                                     ps_y[:, bt * P:(bt + 1) * P],
                                     mybir.ActivationFunctionType.Identity,
                                     bias=b2t[:])
                ps_t = psum_y_pool.tile([P, DY], f32, tag="psy")
                nc.tensor.transpose(
                    ps_t[:], ytb[:, bt * P:(bt + 1) * P], ident[:DY, :DY])
                nc.vector.tensor_copy(y_sb[:, bt, :], ps_t[:])
            nc.sync.dma_start(
                y_d.ap().rearrange("(bt p) j -> p bt j", p=P), y_sb[:])

    nc.compile()
    return nc


def _tiles_pk(m: np.ndarray) -> np.ndarray:
    """[nt*128, C] -> [128, nt*C] partition-tiled layout (row r = kt*128+p)."""
    nt = m.shape[0] // P
    return np.ascontiguousarray(m.reshape(nt, P, -1).swapaxes(0, 1)).reshape(P, -1)


def kernel(x, A, W1, b1, W2, b2, n_steps) -> np.ndarray:
    x = np.asarray(x, dtype=np.float32)
    A = np.asarray(A, dtype=np.float32)
    W1 = np.asarray(W1, dtype=np.float32)
    b1 = np.asarray(b1, dtype=np.float32)
    W2 = np.asarray(W2, dtype=np.float32)
    b2 = np.asarray(b2, dtype=np.float32)
    n = int(np.asarray(n_steps))

    if n not in _BUILD_CACHE:
        _BUILD_CACHE[n] = _build(n)
    nc = _BUILD_CACHE[n]

    dt = np.float32(1.0 / n) if n > 0 else np.float32(0.0)
    t0 = _tiles_pk(np.ascontiguousarray(dt * A.T, dtype=np.float32))
    w1t = _tiles_pk(np.ascontiguousarray(W1.T))           # [512, 2048]
    w2t = _tiles_pk(np.ascontiguousarray(W2.T))           # [2048, 10]
    b1t = np.ascontiguousarray(b1.reshape(HT, P).T)       # [128, 16]
    b2t = np.ascontiguousarray(b2.reshape(DY, 1))
    ident = np.eye(P, dtype=np.float32)

    in_maps = []
    for c in range(NCORES):
        xs = x[c * BC:(c + 1) * BC, :]                    # [512, 512]
        xt = _tiles_pk(np.ascontiguousarray(xs.T))        # [128, 4*512]
        in_maps.append({
            "xt": xt, "xtr": xt, "t0": t0, "w1t": w1t, "b1t": b1t,
            "w2t": w2t, "b2t": b2t, "ident": ident, "identr": ident,
        })

    trace = bool(os.environ.get("BASS_KERNEL_TRACE"))
    core_ids = list(range(NCORES))
    if trace:
        try:
            res = run_bass_kernel_spmd(nc, in_maps, core_ids, trace=True,
                                       trace_cores=[0])
        except Exception:
            res = run_bass_kernel_spmd(nc, in_maps, core_ids)
    else:
        res = run_bass_kernel_spmd(nc, in_maps, core_ids)
    if trace and res.exec_time_ns is not None:
        print(f"HW exec time: {res.exec_time_ns} ns")

    y = np.concatenate([res.results[c]["y"] for c in range(NCORES)], axis=0)
    return y.astype(np.float32)
